# revision 32
# baseline (speedup 1.0000x reference)
"""BlockDrop ResNet kernel for Trainium2 (8 NeuronCores, data parallel).

Network: seed conv3x3 (3->64) + ReLU, then 3 groups of 5 residual blocks
(g0: 64ch @64x64, g1: 128ch @32x32, g2: 256ch @16x16; first block of each
group strided with 1x1-downsample residual), each block gated per-sample by
policy = (probs >= 0.5); then global average pool + FC to 1000 classes.

Key optimizations over the direct per-sample formulation:
- g0 (and the seed conv) pack TWO samples per matmul with block-diagonal
  weights: sample A's 64 channels live on partitions 0:64, sample B's on
  64:128, so the K=64 convs still use the full 128-wide PE array.
- The program is specialized to the actual policy: a block's convs are
  emitted only if ANY of the samples assigned to that slot (across all 8
  SPMD cores; 16 samples for paired g0 slots) keeps the block.  A host-side
  local-search assigner clusters samples with similar policies onto the
  same slot to maximize skipped blocks.  Per-sample correctness is kept by
  per-partition mask columns in the gated blend.
- Slots where ALL samples keep a block use a short blend
  (out = relu(conv2 + res + b2): 1 DVE + 1 Act op instead of 3 DVE + 1 Act).
Convs are computed as 9 accumulating matmuls over (dy,dx) shifts with
channels on the partition (contraction) dim, activations stored padded
([C, (H+2)*(W+2)]) in SBUF as float32r.
"""

import numpy as np
from contextlib import ExitStack

import concourse.bass as bass
import concourse.tile as tile
from concourse import mybir
from concourse.bass_utils import run_bass_kernel_spmd
from concourse.vector_clock import ScopedClock

F32 = mybir.dt.float32
F32R = mybir.dt.float32r
U32 = mybir.dt.uint32
AF = mybir.ActivationFunctionType
NCORES = 8
SHIFTS = [(dy, dx) for dy in range(3) for dx in range(3)]
ZERO_EACH = False   # debug: memset every padded tile allocation (CoreSim)

# ---------------------------------------------------------------------------
# Workarounds for this walrus build: TPB instructions may carry at most one
# embedded sem wait. Tile attaches multi-waits both to body instructions and
# to the kernel-tail drain; hoist the excess onto same-engine NOPs.
MAX_WAITS = 1
_wsplit_counter = [0]


def _split_excess_waits(nc, max_waits=MAX_WAITS):
    n_split = 0
    for f in nc.m.functions:
        for bb in f.blocks:
            changed = False
            new = []
            for ins in bb.instructions:
                si = ins.sync_info
                if si is not None and len(si.on_wait) > max_waits:
                    waits = list(si.on_wait)
                    keep = waits[:max_waits]
                    extra = waits[max_waits:]
                    for i in range(0, len(extra), max_waits):
                        _wsplit_counter[0] += 1
                        nop = mybir.InstNoOp(
                            name=f"I-wsplit-{_wsplit_counter[0]}", ins=[], outs=[])
                        nop.engine = ins.engine
                        nop.sync_info = mybir.SyncInfo(
                            on_wait=extra[i:i + max_waits], on_update=[])
                        new.append(nop)
                        n_split += 1
                    ins.sync_info = mybir.SyncInfo(
                        on_wait=keep, on_update=list(si.on_update))
                    changed = True
                new.append(ins)
            if changed:
                bb.instructions = new
    return n_split


def _patched_drain_and_barrier(self, tick_clock, wait_clock):
    nc = self.nc
    probe = nc.sync.nop(nofuse=True, hint="tail_drain_waits")
    wait_clock.add_sem_waits(
        probe.ins, ScopedClock({None: tick_clock.global_clock}))
    si = probe.ins.sync_info
    waits = list(si.on_wait) if si is not None else []
    probe.ins.sync_info = mybir.SyncInfo(on_wait=waits[:1], on_update=[])
    for i in range(1, len(waits)):
        n2 = nc.sync.nop(nofuse=True, hint=f"tail_drain_waits_{i}")
        n2.ins.sync_info = mybir.SyncInfo(on_wait=waits[i:i + 1], on_update=[])
    nc.sync.drain()
    nc.all_engine_barrier()
    assert self.sems is not None
    popped = nc._tile_sem_poison_stack.pop()
    assert popped is self._sem_poison
    nc.clear_and_free_semaphores(list(self.sems.allocated().values()))
    nc.all_engine_barrier()


tile.TileContext._drain_and_barrier = _patched_drain_and_barrier


# ---------------------------------------------------------------------------
def r3(ap, c):
    return ap.rearrange("p (r c) -> p r c", c=c)


def build_program(n, ku0, ka0, ku1, ka1, ku2, ka2):
    """Build the SPMD program for n samples per core (n even).

    ku0[j][b]: pair-slot j computes g0 block b (any of its 16 samples keeps).
    ka0[j][b]: ALL of its samples keep (short blend).  ku1/ka1, ku2/ka2: the
    same per slot s (8 samples) for g1/g2.
    """
    assert n % 2 == 0
    npair = n // 2
    nc = bass.Bass()

    P = {}

    def param(name, shape, dt=F32R, out=False):
        P[name] = nc.declare_dram_parameter(name, list(shape), dt, isOutput=out)
        return P[name]

    param("x", [n, 3, 64, 64])
    param("seed_wBD", [64, 128])
    param("seed_b", [128, 1], F32)
    param("g0_dsBD", [128, 128])
    param("g0_w1BD", [128, 1152])
    param("g0_w2BD", [128, 1152])
    param("g0_rw1BD", [4, 128, 1152])
    param("g0_rw2BD", [4, 128, 1152])
    param("g0_dsb", [128, 1], F32)
    param("g0_b1c", [128, 5], F32)
    param("g1_ds_wT", [128, 128])
    param("g1_w1T", [128, 1152])
    param("g1_w2T", [128, 1152])
    param("g1_rw1T", [4, 128, 1152])
    param("g1_rw2T", [4, 128, 1152])
    param("g1_dsb", [128, 1], F32)
    param("g1_b1c", [128, 5], F32)
    param("g2_ds_wT", [128, 256])
    param("g2_w1T", [128, 2304])
    param("g2_w2T", [128, 4608])
    param("g2_rw1T", [4, 128, 4608])
    param("g2_rw2T", [4, 128, 4608])
    param("g2_dsb", [128, 2], F32)
    param("g2_b1c", [128, 10], F32)
    param("g0_m", [128, 5 * npair], F32)
    param("g0_mb2", [128, 5 * npair], F32)
    param("g0_bn", [128, 5 * npair], F32)
    for g in (1, 2):
        param(f"g{g}_m", [128, 5 * n], F32)
        param(f"g{g}_bn", [128, 5 * n], F32)
    param("g1_mb2", [128, 5 * n], F32)
    param("g2_mb2", [128, 2 * 5 * n], F32)
    param("fc_wT", [128, 2000])
    param("fcb", [n, 1000], F32)
    param("y", [n, 1000], F32, out=True)

    stage1 = nc.dram_tensor("stage1", [n, 128, 34 * 34], F32R)

    def pad_tile(pool, shape, tag):
        t = pool.tile(shape, F32R, tag=tag, name=tag)
        if ZERO_EACH:
            nc.gpsimd.memset(t[:].bitcast(U32), 0)
        return t

    with tile.TileContext(nc) as tc, ExitStack() as ctx:
        cpool = ctx.enter_context(tc.tile_pool(name="const", bufs=1))
        pp = ctx.enter_context(tc.tile_pool(name="psum", bufs=6, space="PSUM"))
        ppd = ctx.enter_context(tc.tile_pool(name="psumd", bufs=2, space="PSUM"))

        def ctile(pname, shape=None, dt=F32):
            t = cpool.tile(shape or [s for s in P[pname].shape], dt,
                           tag=pname, name=pname)
            nc.sync.dma_start(t[:], P[pname][:])
            return t

        # constants resident for the whole program
        seed_w = cpool.tile([64, 128], F32R, tag="seed_wBD", name="seed_wBD")
        nc.sync.dma_start(seed_w[:], P["seed_wBD"][:])
        seed_b = ctile("seed_b")
        pooled = [cpool.tile([128, n], F32R, tag=f"pooled{k}", name=f"pooled{k}")
                  for k in range(2)]

        # ------------------------------------------------------------------
        # Stage 1: per pair, seed conv + g0 (5 paired blocks) + g1 per sample
        # (b0 stride-2 from the g0 output, then 4 identity blocks); final g1
        # activation (padded, f32r) staged to DRAM.  Emission interleaves
        # pair j's g1 with pair j+1's g0 (and g1's two samples with each
        # other) so the PE never drains at block boundaries.
        with ExitStack() as s1:
            wres = s1.enter_context(tc.tile_pool(name="wres", bufs=1))
            wstr = s1.enter_context(tc.tile_pool(name="wstr", bufs=3))
            patchp = s1.enter_context(tc.tile_pool(name="patch", bufs=2))
            g0genp = s1.enter_context(tc.tile_pool(name="g0gen", bufs=3))
            g0hp = s1.enter_context(tc.tile_pool(name="g0h", bufs=1))
            g1genp = s1.enter_context(tc.tile_pool(name="g1gen", bufs=4))
            g1hp = s1.enter_context(tc.tile_pool(name="g1h", bufs=2))
            tmpp = s1.enter_context(tc.tile_pool(name="tmp", bufs=2))
            resp = s1.enter_context(tc.tile_pool(name="res", bufs=1))
            res1p = s1.enter_context(tc.tile_pool(name="res1", bufs=2))

            xflat = P["x"].rearrange("n c h w -> (n c) h w")
            patch_views = {}

            def emit_patch(j):
                pt = pad_tile(patchp, [64, 64 * 64], "patch")
                vpt = r3(pt[:], 64)
                engs = [nc.sync, nc.scalar] if j == 0 else [nc.sync, nc.gpsimd]
                for si, (dy, dx) in enumerate(SHIFTS):
                    rr0, rr1 = max(0, 1 - dy), min(64, 65 - dy)
                    cc0, cc1 = max(0, 1 - dx), min(64, 65 - dx)
                    engs[si % 2].dma_start(
                        vpt[6 * si:6 * si + 6, rr0:rr1, cc0:cc1],
                        xflat[6 * j:6 * j + 6, rr0 + dy - 1:rr1 + dy - 1,
                              cc0 + dx - 1:cc1 + dx - 1])
                patch_views[j] = vpt

            def wt_res(pname):
                t = wres.tile([s for s in P[pname].shape], F32R,
                              tag=pname, name=pname)
                nc.sync.dma_start(t[:], P[pname][:])
                return t

            g0_ds = wt_res("g0_dsBD")
            g1_ds = wt_res("g1_ds_wT")
            g1_w1 = wt_res("g1_w1T")
            g1_w2 = wt_res("g1_w2T")
            g0_dsb = ctile("g0_dsb")
            g0_b1c = ctile("g0_b1c")
            latec = {}

            def emit_late_consts():
                for nm in ("g0_m", "g0_mb2", "g0_bn", "g1_m", "g1_mb2",
                           "g1_bn", "g1_b1c", "g1_dsb", "g2_m", "g2_mb2",
                           "g2_bn", "g2_b1c", "g2_dsb"):
                    latec[nm] = ctile(nm)

            def wt_str(pname, j, tag, eng=None):
                t = wstr.tile([128, 1152], F32R, tag=tag, name=f"{tag}{j}")
                src = P[pname] if j is None else P[pname][j]
                (eng or nc.sync).dma_start(t[:], src[:])
                return t

            # pre-zero the rotating padded buffers once (borders stay zero:
            # every later generation writes only the interior cells)
            def prezero(pool, bufs, shape, tag):
                for _ in range(bufs):
                    t = pool.tile(shape, F32R, tag=tag, name=tag)
                    nc.gpsimd.memset(t[:].bitcast(U32), 0)

            prezero(patchp, 2, [64, 64 * 64], "patch")
            emit_patch(0)
            preload = {}
            preload[("w1", 0)] = wt_str("g0_w1BD", None, "rw1", nc.scalar)
            preload[("w2", 0)] = wt_str("g0_w2BD", None, "rw2", nc.scalar)
            prezero(g0genp, 3, [128, 66 * 66], "g0x")
            prezero(g0hp, 1, [128, 66 * 66], "g0h")
            prezero(g1genp, 4, [128, 34 * 34], "g1x")
            prezero(g1hp, 2, [128, 34 * 34], "g1h")

            def blend(ps2, res_ap, out_ap, mcol, mb2col, bncol, C, allkeep):
                """out = Relu(m*(ps2+res) + m*b2) + (1-m)*res (views [p,R,C]).
                allkeep: every gated sample has m=1 -> out = Relu(t + b2)."""
                R = ps2.shape[1]
                t = r3(tmpp.tile([128, R * C], F32, tag="t", name="t")[:], C)
                nc.vector.tensor_add(t, ps2, res_ap)
                if allkeep:
                    nc.scalar.activation(out_ap, t, AF.Relu, bias=mb2col,
                                         scale=1.0)
                    return
                rm = r3(tmpp.tile([128, R * C], F32, tag="rm", name="rm")[:], C)
                nc.scalar.activation(rm, t, AF.Relu, bias=mb2col, scale=mcol)
                rb = r3(tmpp.tile([128, R * C], F32, tag="rb", name="rb")[:], C)
                nc.scalar.mul(rb, res_ap, bncol)
                nc.vector.tensor_add(out_ap, rm, rb)

            # ---- unit builders.  A unit is (est_pe_rows, closure); closures
            # allocate tiles and emit instructions when called, so emission
            # order == engine queue order.  State flows through per-pair dicts.
            def g0_units(j):
                st = {}
                units = []

                def u_seed():
                    vpt = patch_views[j]
                    xg = pad_tile(g0genp, [128, 66 * 66], "g0x")
                    vg = r3(xg[:], 66)
                    for c in range(8):
                        ps = pp.tile([128, 512], F32, tag="ps", name="ps")
                        r0 = c * 8
                        nc.tensor.matmul(ps[:], seed_w[0:64, :],
                                         vpt[0:64, r0:r0 + 8, 0:64])
                        ov = vg[:, 1 + r0:1 + r0 + 8, 1:65]
                        if c % 2 == 0:
                            nc.scalar.activation(ov, r3(ps[:], 64), AF.Relu,
                                                 bias=seed_b[:, 0:1], scale=1.0)
                        else:
                            nc.vector.tensor_scalar(
                                ov, r3(ps[:], 64), seed_b[:, 0:1], 0.0,
                                mybir.AluOpType.add, mybir.AluOpType.max)
                    st["vg"] = vg

                units.append((4096, u_seed))
                if j + 1 < npair:
                    units.append((0, lambda: emit_patch(j + 1)))

                def u_ds(keep):
                    def fn():
                        vg = st["vg"]
                        if keep:
                            restile = resp.tile([128, 4096], F32, tag="g0res",
                                                name="g0res")
                            st["res"] = restile
                        else:
                            xg2 = pad_tile(g0genp, [128, 66 * 66], "g0x")
                            vg2 = r3(xg2[:], 66)
                        for c in range(8):
                            psd = pp.tile([128, 512], F32, tag="ps",
                                          name="ps")
                            nc.tensor.matmul(psd[:], g0_ds[:],
                                             vg[:, c * 8 + 1:c * 8 + 9, 1:65])
                            if keep:
                                ov, iv = restile[:, c * 512:(c + 1) * 512], psd[:]
                            else:
                                ov, iv = (vg2[:, c * 8 + 1:c * 8 + 9, 1:65],
                                          r3(psd[:], 64))
                            if c % 2 == 0:
                                nc.scalar.activation(
                                    ov, iv, AF.Identity, bias=g0_dsb[:, 0:1],
                                    scale=1.0)
                            else:
                                nc.vector.tensor_scalar_add(ov, iv,
                                                            g0_dsb[:, 0:1])
                        if not keep:
                            st["vg"] = vg2

                    return fn

                def u_conv1(b):
                    def fn():
                        if b == 0:
                            w1 = preload.pop(("w1", j), None) or \
                                wt_str("g0_w1BD", None, "rw1")
                        else:
                            w1 = wt_str("g0_rw1BD", b - 1, "rw1")
                        vg = st["vg"]
                        h = pad_tile(g0hp, [128, 66 * 66], "g0h")
                        vh = r3(h[:], 66)
                        b1col = g0_b1c[:, b:b + 1]
                        for c in range(8):
                            ps1 = pp.tile([128, 512], F32, tag="ps", name="ps")
                            for si, (dy, dx) in enumerate(SHIFTS):
                                nc.tensor.matmul(
                                    ps1[:], w1[:, si * 128:(si + 1) * 128],
                                    vg[:, c * 8 + dy:c * 8 + dy + 8, dx:dx + 64],
                                    start=(si == 0), stop=(si == 8))
                            nc.scalar.activation(
                                vh[:, 1 + c * 8:1 + c * 8 + 8, 1:65],
                                r3(ps1[:], 64), AF.Relu, bias=b1col, scale=1.0)
                        st["vh"] = vh

                    return fn

                def u_conv2(b):
                    def fn():
                        if b == 0:
                            w2 = preload.pop(("w2", j), None) or \
                                wt_str("g0_w2BD", None, "rw2")
                        else:
                            w2 = wt_str("g0_rw2BD", b - 1, "rw2")
                        vg, vh = st["vg"], st["vh"]
                        col = b * npair + j
                        mcol = latec["g0_m"][:, col:col + 1]
                        mb2col = latec["g0_mb2"][:, col:col + 1]
                        bncol = latec["g0_bn"][:, col:col + 1]
                        xg2 = pad_tile(g0genp, [128, 66 * 66], "g0x")
                        vg2 = r3(xg2[:], 66)
                        for c in range(8):
                            ps2 = pp.tile([128, 512], F32, tag="ps", name="ps")
                            for si, (dy, dx) in enumerate(SHIFTS):
                                nc.tensor.matmul(
                                    ps2[:], w2[:, si * 128:(si + 1) * 128],
                                    vh[:, c * 8 + dy:c * 8 + dy + 8, dx:dx + 64],
                                    start=(si == 0), stop=(si == 8))
                            if b == 0:
                                res_ap = r3(st["res"][:, c * 512:(c + 1) * 512],
                                            64)
                            else:
                                res_ap = vg[:, 1 + c * 8:1 + c * 8 + 8, 1:65]
                            blend(r3(ps2[:], 64), res_ap,
                                  vg2[:, 1 + c * 8:1 + c * 8 + 8, 1:65],
                                  mcol, mb2col, bncol, 64, ka0[j][b])
                        st["vg"] = vg2

                    return fn

                units.append((4096, u_ds(ku0[j][0])))
                first_conv = True
                for b in range(5):
                    if ku0[j][b]:
                        units.append((36864, u_conv1(b)))
                        if j == 0 and first_conv:
                            units.append((0, emit_late_consts))
                            first_conv = False
                        units.append((36864, u_conv2(b)))
                if j == 0 and first_conv:
                    units.append((0, emit_late_consts))
                st_holder[j] = st
                return units

            def g1_units_sample(j, half, s):
                st = {}
                units = []
                keep0 = ku1[s][0]

                def u_b0c1():
                    vg = st_holder[j]["vg"]
                    x1 = pad_tile(g1genp, [128, 34 * 34], "g1x")
                    vx1 = r3(x1[:], 34)
                    st["vx1"] = vx1
                    st["x1"] = x1
                    if keep0:
                        res1 = res1p.tile([128, 1024], F32, tag="g1res",
                                          name="g1res")
                        st["res1"] = res1
                        h1 = pad_tile(g1hp, [128, 34 * 34], "g1h")
                        vh1 = r3(h1[:], 34)
                        st["vh1"] = vh1
                    for c in range(2):
                        psd = ppd.tile([128, 512], F32, tag="psds", name="psds")
                        nc.tensor.matmul(
                            psd[:], g1_ds[half:half + 64, :],
                            vg[half:half + 64,
                               1 + 32 * c:1 + 32 * c + 32:2, 1:65:2])
                        if keep0:
                            nc.scalar.activation(
                                res1[:, c * 512:(c + 1) * 512], psd[:],
                                AF.Identity, bias=latec["g1_dsb"][:, 0:1], scale=1.0)
                            ps1 = pp.tile([128, 512], F32, tag="ps", name="ps")
                            for si, (dy, dx) in enumerate(SHIFTS):
                                nc.tensor.matmul(
                                    ps1[:],
                                    g1_w1[half:half + 64,
                                          si * 128:(si + 1) * 128],
                                    vg[half:half + 64,
                                       dy + 32 * c:dy + 32 * c + 32:2,
                                       dx:dx + 64:2],
                                    start=(si == 0), stop=(si == 8))
                            nc.scalar.activation(
                                vh1[:, 1 + 16 * c:1 + 16 * c + 16, 1:33],
                                r3(ps1[:], 32),
                                AF.Relu, bias=latec["g1_b1c"][:, 0:1], scale=1.0)
                        else:
                            nc.scalar.activation(
                                vx1[:, 1 + 16 * c:1 + 16 * c + 16, 1:33],
                                r3(psd[:], 32),
                                AF.Identity, bias=latec["g1_dsb"][:, 0:1], scale=1.0)

                def u_b0c2():
                    mcol = latec["g1_m"][:, s:s + 1]
                    mb2col = latec["g1_mb2"][:, s:s + 1]
                    bncol = latec["g1_bn"][:, s:s + 1]
                    vx1, vh1 = st["vx1"], st["vh1"]
                    res1 = st["res1"]
                    for c in range(2):
                        ps2 = pp.tile([128, 512], F32, tag="ps", name="ps")
                        for si, (dy, dx) in enumerate(SHIFTS):
                            nc.tensor.matmul(
                                ps2[:], g1_w2[:, si * 128:(si + 1) * 128],
                                vh1[:, dy + 16 * c:dy + 16 * c + 16, dx:dx + 32],
                                start=(si == 0), stop=(si == 8))
                        blend(r3(ps2[:], 32),
                              r3(res1[:, c * 512:(c + 1) * 512], 32),
                              vx1[:, 1 + 16 * c:1 + 16 * c + 16, 1:33],
                              mcol, mb2col, bncol, 32, ka1[s][0])

                units.append((10240 if keep0 else 1024, u_b0c1))
                if keep0:
                    units.append((9216, u_b0c2))

                def u_c1(b):
                    def fn():
                        w1 = wt_str("g1_rw1T", b - 1, "rw1")
                        vx1 = st["vx1"]
                        h1 = pad_tile(g1hp, [128, 34 * 34], "g1h")
                        vh1 = r3(h1[:], 34)
                        st["vh1"] = vh1
                        for c in range(2):
                            ps1 = pp.tile([128, 512], F32, tag="ps", name="ps")
                            for si, (dy, dx) in enumerate(SHIFTS):
                                nc.tensor.matmul(
                                    ps1[:], w1[:, si * 128:(si + 1) * 128],
                                    vx1[:, dy + 16 * c:dy + 16 * c + 16,
                                        dx:dx + 32],
                                    start=(si == 0), stop=(si == 8))
                            nc.scalar.activation(
                                vh1[:, 1 + 16 * c:1 + 16 * c + 16, 1:33],
                                r3(ps1[:], 32), AF.Relu,
                                bias=latec["g1_b1c"][:, b:b + 1], scale=1.0)

                    return fn

                def u_c2(b):
                    def fn():
                        w2 = wt_str("g1_rw2T", b - 1, "rw2")
                        mi = b * n + s
                        mcol = latec["g1_m"][:, mi:mi + 1]
                        mb2col = latec["g1_mb2"][:, mi:mi + 1]
                        bncol = latec["g1_bn"][:, mi:mi + 1]
                        vx1, vh1 = st["vx1"], st["vh1"]
                        x1n = pad_tile(g1genp, [128, 34 * 34], "g1x")
                        vx1n = r3(x1n[:], 34)
                        for c in range(2):
                            ps2 = pp.tile([128, 512], F32, tag="ps", name="ps")
                            for si, (dy, dx) in enumerate(SHIFTS):
                                nc.tensor.matmul(
                                    ps2[:], w2[:, si * 128:(si + 1) * 128],
                                    vh1[:, dy + 16 * c:dy + 16 * c + 16,
                                        dx:dx + 32],
                                    start=(si == 0), stop=(si == 8))
                            blend(r3(ps2[:], 32),
                                  vx1[:, 1 + 16 * c:1 + 16 * c + 16, 1:33],
                                  vx1n[:, 1 + 16 * c:1 + 16 * c + 16, 1:33],
                                  mcol, mb2col, bncol, 32, ka1[s][b])
                        st["vx1"], st["x1"] = vx1n, x1n

                    return fn

                for b in range(1, 5):
                    if ku1[s][b]:
                        units.append((9216, u_c1(b)))
                        units.append((9216, u_c2(b)))
                units.append((0, lambda: nc.sync.dma_start(stage1[s],
                                                           st["x1"][:])))
                return units

            def merge_units(a, b):
                out = []
                ia = ib = 0
                ta = tb = 0
                while ia < len(a) or ib < len(b):
                    if ib >= len(b) or (ia < len(a) and ta <= tb):
                        est, fn = a[ia]
                        ia += 1
                        ta += est
                        out.append((est, fn))
                    else:
                        est, fn = b[ib]
                        ib += 1
                        tb += est
                        out.append((est, fn))
                return out

            st_holder = {}
            prev_g1 = []
            for j in range(npair):
                for est, fn in merge_units(prev_g1, g0_units(j)):
                    fn()
                prev_g1 = merge_units(
                    g1_units_sample(j, 0, 2 * j),
                    g1_units_sample(j, 64, 2 * j + 1))
            for est, fn in prev_g1:
                fn()

        # ------------------------------------------------------------------
        # Stage 2: g2, block-outer over all samples; then pool + fc.
        with ExitStack() as s2:
            w2p = s2.enter_context(tc.tile_pool(name="wg2", bufs=2))
            g2inp = s2.enter_context(tc.tile_pool(name="g2in", bufs=3))
            g2genp = s2.enter_context(tc.tile_pool(name="g2gen", bufs=n + 4))
            g2hp = s2.enter_context(tc.tile_pool(name="g2h", bufs=3))
            tmp2p = s2.enter_context(tc.tile_pool(name="tmp2", bufs=3))

            for _ in range(n + 4):
                t = g2genp.tile([128, 2 * 324], F32R, tag="g2x", name="g2x")
                nc.gpsimd.memset(t[:].bitcast(U32), 0)
            for _ in range(3):
                t = g2hp.tile([128, 2 * 324], F32R, tag="g2h", name="g2h")
                nc.gpsimd.memset(t[:].bitcast(U32), 0)

            def g2_conv(ps_pair, w, views, nshift=9):
                nki = len(views)
                for ki in range(nki):
                    for si in range(nshift):
                        for mo in range(2):
                            col = ki * nshift * 256 + si * 256 + mo * 128
                            nc.tensor.matmul(
                                ps_pair[mo][:], w[:, col:col + 128],
                                views[ki][si],
                                start=(ki == 0 and si == 0),
                                stop=(ki == nki - 1 and si == nshift - 1))

            cur = [None] * n  # current g2 activation tile per sample slot
            lastb = [max([0] + [b for b in range(5) if ku2[s][b]])
                     for s in range(n)]

            def emit_pool(s):
                vxg = [r3(cur[s][:, ki * 324:(ki + 1) * 324], 18)
                       for ki in range(2)]
                with nc.allow_low_precision(reason="pooled stored f32r"):
                    for ki in range(2):
                        nc.vector.reduce_sum(pooled[ki][:, s:s + 1],
                                             vxg[ki][:, 1:17, 1:17],
                                             axis=mybir.AxisListType.XY)

            for b in range(5):
                any_b = any(ku2[s][b] for s in range(n)) or b == 0
                if any_b and b == 0:
                    dsT = w2p.tile([128, 256], F32R, tag="g2ds", name="g2ds")
                    nc.scalar.dma_start(dsT[:], P["g2_ds_wT"][:])
                    w1t = w2p.tile([128, 2304], F32R, tag="g2w1", name="g2w1")
                    nc.scalar.dma_start(w1t[:], P["g2_w1T"][:])
                    w2t = w2p.tile([128, 4608], F32R, tag="g2w2", name="g2w2")
                    nc.scalar.dma_start(w2t[:], P["g2_w2T"][:])
                elif any_b and b > 0:
                    w1t = w2p.tile([128, 4608], F32R, tag="g2w1", name="g2w1")
                    nc.sync.dma_start(w1t[:], P["g2_rw1T"][b - 1])
                    w2t = w2p.tile([128, 4608], F32R, tag="g2w2", name="g2w2")
                    nc.sync.dma_start(w2t[:], P["g2_rw2T"][b - 1])

                def part1(b, s, w1t, dsT):
                    """ds (b0) + conv1 + h2 acts; returns state for part2, or
                    None if fully handled (dropped b0)."""
                    if b == 0:
                        x1t = g2inp.tile([128, 34 * 34], F32R, tag="g2in",
                                         name="g2in")
                        (nc.scalar if s % 2 == 0 else nc.sync).dma_start(
                            x1t[:], stage1[s])
                        vin = r3(x1t[:], 34)
                        if not ku2[s][0]:
                            nxt = pad_tile(g2genp, [128, 2 * 324], "g2x")
                            vnxt = [r3(nxt[:, ki * 324:(ki + 1) * 324], 18)
                                    for ki in range(2)]
                            for mo in range(2):
                                psd = ppd.tile([128, 256], F32, tag="psds",
                                               name="psds")
                                nc.tensor.matmul(
                                    psd[:], dsT[:, mo * 128:(mo + 1) * 128],
                                    vin[:, 1:33:2, 1:33:2])
                                nc.scalar.activation(
                                    vnxt[mo][:, 1:17, 1:17], r3(psd[:], 16),
                                    AF.Identity,
                                    bias=latec["g2_dsb"][:, mo:mo + 1],
                                    scale=1.0)
                            cur[s] = nxt
                            if lastb[s] == 0:
                                emit_pool(s)
                            return None
                        restmp = tmp2p.tile([128, 512], F32, tag="res2",
                                            name="res2")
                        for mo in range(2):
                            psd = ppd.tile([128, 256], F32, tag="psds",
                                           name="psds")
                            nc.tensor.matmul(
                                psd[:], dsT[:, mo * 128:(mo + 1) * 128],
                                vin[:, 1:33:2, 1:33:2])
                            nc.scalar.activation(
                                restmp[:, mo * 256:(mo + 1) * 256], psd[:],
                                AF.Identity, bias=latec["g2_dsb"][:, mo:mo + 1],
                                scale=1.0)
                        res_aps = [r3(restmp[:, mo * 256:(mo + 1) * 256], 16)
                                   for mo in range(2)]
                        w1_views = [[vin[:, dy:dy + 32:2, dx:dx + 32:2]
                                     for (dy, dx) in SHIFTS]]
                    else:
                        xg = cur[s]
                        vxg = [r3(xg[:, ki * 324:(ki + 1) * 324], 18)
                               for ki in range(2)]
                        res_aps = [vxg[mo][:, 1:17, 1:17] for mo in range(2)]
                        w1_views = [[vxg[ki][:, dy:dy + 16, dx:dx + 16]
                                     for (dy, dx) in SHIFTS] for ki in range(2)]
                    h2 = pad_tile(g2hp, [128, 2 * 324], "g2h")
                    vh2 = [r3(h2[:, ki * 324:(ki + 1) * 324], 18)
                           for ki in range(2)]
                    ps1p = [pp.tile([128, 256], F32, tag="ps", name="ps")
                            for _ in range(2)]
                    g2_conv(ps1p, w1t, w1_views)
                    for mo in range(2):
                        nc.scalar.activation(
                            vh2[mo][:, 1:17, 1:17], r3(ps1p[mo][:], 16),
                            AF.Relu,
                            bias=latec["g2_b1c"][:, 2 * b + mo:2 * b + mo + 1],
                            scale=1.0)
                    return {"vh2": vh2, "res_aps": res_aps}

                def part2(b, s, st, w2t):
                    mcol = latec["g2_m"][:, b * n + s:b * n + s + 1]
                    bncol = latec["g2_bn"][:, b * n + s:b * n + s + 1]
                    mb2c = [latec["g2_mb2"][:, (b * n + s) * 2 + mo:
                                            (b * n + s) * 2 + mo + 1]
                            for mo in range(2)]
                    vh2, res_aps = st["vh2"], st["res_aps"]
                    ps2p = [pp.tile([128, 256], F32, tag="ps", name="ps")
                            for _ in range(2)]
                    g2_conv(ps2p, w2t,
                            [[vh2[ki][:, dy:dy + 16, dx:dx + 16]
                              for (dy, dx) in SHIFTS] for ki in range(2)])
                    nxt = pad_tile(g2genp, [128, 2 * 324], "g2x")
                    vnxt = [r3(nxt[:, ki * 324:(ki + 1) * 324], 18)
                            for ki in range(2)]
                    for mo in range(2):
                        t2 = r3(tmp2p.tile([128, 256], F32, tag="t2",
                                           name="t2")[:], 16)
                        nc.vector.tensor_add(t2, r3(ps2p[mo][:], 16),
                                             res_aps[mo])
                        if ka2[s][b]:
                            nc.scalar.activation(vnxt[mo][:, 1:17, 1:17], t2,
                                                 AF.Relu, bias=mb2c[mo],
                                                 scale=1.0)
                            continue
                        rm = r3(tmp2p.tile([128, 256], F32, tag="rm2",
                                           name="rm2")[:], 16)
                        nc.scalar.activation(rm, t2, AF.Relu,
                                             bias=mb2c[mo], scale=mcol)
                        rb = r3(tmp2p.tile([128, 256], F32, tag="rb2",
                                           name="rb2")[:], 16)
                        nc.vector.tensor_scalar_mul(rb, res_aps[mo], bncol)
                        nc.vector.tensor_add(vnxt[mo][:, 1:17, 1:17], rm, rb)
                    cur[s] = nxt
                    if lastb[s] == b:
                        emit_pool(s)

                prev = None
                for s in range(n):
                    if not ku2[s][b] and b > 0:
                        continue
                    st = part1(b, s, w1t, dsT if b == 0 else None)
                    if st is None:
                        continue
                    if prev is not None:
                        part2(b, prev[0], prev[1], w2t)
                    prev = (s, st)
                if prev is not None:
                    part2(b, prev[0], prev[1], w2t)

            # fc: y[s, k] = pooled[:, s] . fc_wT[:, k]
            fcw = w2p.tile([128, 2000], F32R, tag="fcw", name="fcw")
            nc.scalar.dma_start(fcw[:], P["fc_wT"][:])
            fcbt = w2p.tile([n, 1000], F32, tag="fcb", name="fcb")
            nc.scalar.dma_start(fcbt[:], P["fcb"][:])
            outsb = w2p.tile([n, 1000], F32, tag="outsb", name="outsb")
            for nb, (n0, n1) in enumerate([(0, 512), (512, 1000)]):
                psf = ppd.tile([n, n1 - n0], F32, tag="psds", name="psds")
                for ki in range(2):
                    nc.tensor.matmul(psf[:], pooled[ki][:, 0:n],
                                     fcw[:, ki * 1000 + n0:ki * 1000 + n1],
                                     start=(ki == 0), stop=(ki == 1))
                nc.vector.tensor_add(outsb[:, n0:n1], psf[:], fcbt[:, n0:n1])
            nc.sync.dma_start(P["y"][:], outsb[:])

    _split_excess_waits(nc)
    return nc


# ---------------------------------------------------------------------------
# Host-side: sample-to-slot assignment optimization.  Minimizes the SPMD
# union-keep compute: a g0 block is computed for a pair-slot if any of its 16
# samples (2 slots x 8 cores) keeps it; g1/g2 blocks per slot (8 samples).
W_G0, W_G1, W_G2B0, W_G2 = 73728, 18432, 13824, 18432


def optimize_assignment(pol, ncores, n, iters=400000, seeds=6):
    """Best-of-N restarts of a swap hill-climb."""
    best = None
    best_cost = None
    for seed in range(seeds):
        A, c = _optimize_assignment_once(pol, ncores, n, iters, seed)
        if best_cost is None or c < best_cost:
            best, best_cost = A, c
    return best


def _optimize_assignment_once(pol, ncores, n, iters, seed):
    npair = n // 2
    rng = np.random.default_rng(seed)
    key = pol[:, 0:5] @ (2 ** np.arange(5)[::-1]) * 1024 + \
        pol[:, 5:15] @ (2 ** np.arange(10)[::-1])
    order = np.argsort(key, kind="stable")
    A = np.zeros((ncores, n), dtype=np.int64)
    for j in range(npair):
        blk = order[2 * ncores * j:2 * ncores * (j + 1)]
        A[:, 2 * j] = blk[:ncores]
        A[:, 2 * j + 1] = blk[ncores:]

    # bitmask encoding: bits 0-4 g0, 5-9 g1, 10-14 g2 keep flags
    pat = [int(v) for v in (pol.astype(np.uint32) @
                            (1 << np.arange(15, dtype=np.uint32)))]
    pop = [bin(m).count("1") for m in range(32)]
    cost_pair = [W_G0 * pop[m & 31] for m in range(32768)]
    cost_slot = [W_G1 * pop[(m >> 5) & 31] + W_G2 * pop[(m >> 11) & 15]
                 + W_G2B0 * ((m >> 10) & 1) for m in range(32768)]
    cols = [[pat[A[c, s]] for c in range(ncores)] for s in range(n)]

    def slot_u(col):
        u = 0
        for v in col:
            u |= v
        return u

    slot_un = [slot_u(cols[s]) for s in range(n)]
    ri = rng.integers(0, ncores, size=2 * iters)
    rs = rng.integers(0, n, size=2 * iters)
    for it in range(iters):
        c0, c1 = ri[2 * it], ri[2 * it + 1]
        s0, s1 = rs[2 * it], rs[2 * it + 1]
        if s0 == s1:
            continue
        j0, j1 = s0 // 2, s1 // 2
        u00, u01 = slot_un[2 * j0], slot_un[2 * j0 + 1]
        u10, u11 = slot_un[2 * j1], slot_un[2 * j1 + 1]
        old = (cost_slot[slot_un[s0]] + cost_slot[slot_un[s1]]
               + cost_pair[u00 | u01] + (cost_pair[u10 | u11]
                                         if j1 != j0 else 0))
        cols[s0][c0], cols[s1][c1] = cols[s1][c1], cols[s0][c0]
        n0, n1 = slot_u(cols[s0]), slot_u(cols[s1])
        nu = {s0: n0, s1: n1}
        p0 = (nu.get(2 * j0, slot_un[2 * j0])
              | nu.get(2 * j0 + 1, slot_un[2 * j0 + 1]))
        p1 = (nu.get(2 * j1, slot_un[2 * j1])
              | nu.get(2 * j1 + 1, slot_un[2 * j1 + 1]))
        new = (cost_slot[n0] + cost_slot[n1]
               + cost_pair[p0] + (cost_pair[p1] if j1 != j0 else 0))
        if new <= old:
            A[c0, s0], A[c1, s1] = A[c1, s1], A[c0, s0]
            slot_un[s0], slot_un[s1] = n0, n1
        else:
            cols[s0][c0], cols[s1][c1] = cols[s1][c1], cols[s0][c0]
    cost = 0
    for s in range(n):
        cost += cost_slot[slot_un[s]]
    for j in range(npair):
        cost += cost_pair[slot_un[2 * j] | slot_un[2 * j + 1]]
    return A, cost


# ---------------------------------------------------------------------------
def _dup(a):
    return np.concatenate([a, a], axis=0)


def _prep_shared(inputs):
    """Weight/bias tensors shared by all cores, in device layout (f32)."""
    f = np.float32
    d = {}
    sw = inputs["seed_w"].astype(f)  # [64, 3, 3, 3]
    swT = np.transpose(sw, (2, 3, 1, 0)).reshape(27, 64)  # [(dy dx ch), o]
    # patch partition layout: row si*6 + smp*3 + c (smp 0 = sample A -> out
    # cols 0:64, smp 1 = B -> cols 64:128)
    z = np.zeros((64, 128), f)
    for si in range(9):
        for c in range(3):
            z[si * 6 + c, 0:64] = swT[si * 3 + c]
            z[si * 6 + 3 + c, 64:128] = swT[si * 3 + c]
    d["seed_wBD"] = z
    d["seed_b"] = _dup(inputs["seed_b"].astype(f).reshape(64, 1))

    def c3bd(w):  # [64,64,3,3] -> [128, 9*128] block-diag per tap
        t = np.transpose(w.astype(f), (1, 2, 3, 0))  # [I, 3, 3, O]
        bd = np.zeros((128, 9 * 128), f)
        for si in range(9):
            blk = t[:, si // 3, si % 3, :]  # [64, 64]
            bd[0:64, si * 128:si * 128 + 64] = blk
            bd[64:128, si * 128 + 64:si * 128 + 128] = blk
        return bd

    dsT = inputs["g0_ds_w"].astype(f).reshape(64, 64).T
    dsbd = np.zeros((128, 128), f)
    dsbd[0:64, 0:64] = dsT
    dsbd[64:128, 64:128] = dsT
    d["g0_dsBD"] = dsbd
    d["g0_w1BD"] = c3bd(inputs["g0_b0_w1"])
    d["g0_w2BD"] = c3bd(inputs["g0_b0_w2"])
    d["g0_rw1BD"] = np.stack([c3bd(inputs["g0_r_w1"][j]) for j in range(4)])
    d["g0_rw2BD"] = np.stack([c3bd(inputs["g0_r_w2"][j]) for j in range(4)])
    d["g0_dsb"] = _dup(inputs["g0_ds_b"].astype(f).reshape(64, 1))
    d["g0_b1c"] = _dup(np.stack(
        [inputs["g0_b0_b1"].astype(f)] +
        [inputs["g0_r_b1"][j].astype(f) for j in range(4)], axis=1))

    def c3(w, dup):  # [O,I,3,3] -> [I(x2 if dup), 9*O]
        O, I = w.shape[0], w.shape[1]
        t = np.transpose(w.astype(f), (1, 2, 3, 0)).reshape(I, 9 * O)
        return _dup(t) if dup else t

    d["g1_ds_wT"] = _dup(inputs["g1_ds_w"].astype(f).reshape(128, 64).T.copy())
    d["g1_w1T"] = c3(inputs["g1_b0_w1"], True)
    d["g1_w2T"] = c3(inputs["g1_b0_w2"], False)
    d["g1_rw1T"] = np.stack([c3(inputs["g1_r_w1"][j], False) for j in range(4)])
    d["g1_rw2T"] = np.stack([c3(inputs["g1_r_w2"][j], False) for j in range(4)])
    d["g1_dsb"] = inputs["g1_ds_b"].astype(f).reshape(128, 1)
    d["g1_b1c"] = np.stack(
        [inputs["g1_b0_b1"].astype(f)] +
        [inputs["g1_r_b1"][j].astype(f) for j in range(4)], axis=1)

    # g2: cols (ki, s, mo, o)
    def g2c3(w):  # [256, I, 3, 3] -> [128, (ki) 9 2 128] with ki blocks of I
        I = w.shape[1]
        t = np.transpose(w.astype(f), (1, 2, 3, 0))  # [I, 3,3, 256]
        t = t.reshape(I // 128, 128, 9, 2, 128)      # [ki, i, s, mo, o]
        t = np.transpose(t, (1, 0, 2, 3, 4)).reshape(128, -1)
        return t

    d["g2_ds_wT"] = inputs["g2_ds_w"].astype(f).reshape(256, 128).T.reshape(
        128, 2, 128).reshape(128, 256)
    d["g2_w1T"] = g2c3(inputs["g2_b0_w1"])
    d["g2_w2T"] = g2c3(inputs["g2_b0_w2"])
    d["g2_rw1T"] = np.stack([g2c3(inputs["g2_r_w1"][j]) for j in range(4)])
    d["g2_rw2T"] = np.stack([g2c3(inputs["g2_r_w2"][j]) for j in range(4)])
    d["g2_dsb"] = inputs["g2_ds_b"].astype(f).reshape(2, 128).T.copy().reshape(
        128, 2)
    b1s = [inputs["g2_b0_b1"].astype(f)] + \
        [inputs["g2_r_b1"][j].astype(f) for j in range(4)]
    d["g2_b1c"] = np.stack([b[mo * 128:(mo + 1) * 128]
                            for b in b1s for mo in range(2)], axis=1)
    d["fc_wT"] = (inputs["fc_w"].astype(f).T / 256.0).reshape(
        2, 128, 1000).transpose(1, 0, 2).reshape(128, 2000)
    return d


def _prep_core(inputs, shared, policy, A, c, n):
    f = np.float32
    npair = n // 2
    sel = A[c]  # global sample index per slot
    d = dict(shared)
    d["x"] = np.ascontiguousarray(inputs["inputs"][sel].astype(f))
    pol = policy[sel]  # [n, 15]

    b2_g0 = [inputs["g0_b0_b2"]] + [inputs["g0_r_b2"][j] for j in range(4)]
    b2_g1 = [inputs["g1_b0_b2"]] + [inputs["g1_r_b2"][j] for j in range(4)]
    b2_g2 = [inputs["g2_b0_b2"]] + [inputs["g2_r_b2"][j] for j in range(4)]

    g0_m = np.zeros((128, 5 * npair), f)
    g0_mb2 = np.zeros((128, 5 * npair), f)
    g0_bn = np.zeros((128, 5 * npair), f)
    for b in range(5):
        bv = np.asarray(b2_g0[b], f)
        for j in range(npair):
            col = b * npair + j
            for half, s in ((0, 2 * j), (64, 2 * j + 1)):
                m = pol[s, b]
                g0_m[half:half + 64, col] = m
                g0_mb2[half:half + 64, col] = m * bv
                g0_bn[half:half + 64, col] = 1 - m
    d["g0_m"], d["g0_mb2"], d["g0_bn"] = g0_m, g0_mb2, g0_bn

    g1_m = np.zeros((128, 5 * n), f)
    g1_mb2 = np.zeros((128, 5 * n), f)
    g1_bn = np.zeros((128, 5 * n), f)
    for b in range(5):
        bv = np.asarray(b2_g1[b], f)
        for s in range(n):
            m = pol[s, 5 + b]
            j = b * n + s
            g1_m[:, j], g1_mb2[:, j], g1_bn[:, j] = m, m * bv, 1 - m
    d["g1_m"], d["g1_mb2"], d["g1_bn"] = g1_m, g1_mb2, g1_bn

    g2_m = np.zeros((128, 5 * n), f)
    g2_mb2 = np.zeros((128, 2 * 5 * n), f)
    g2_bn = np.zeros((128, 5 * n), f)
    for b in range(5):
        bv = np.asarray(b2_g2[b], f)
        for s in range(n):
            m = pol[s, 10 + b]
            j = b * n + s
            g2_m[:, j], g2_bn[:, j] = m, 1 - m
            for mo in range(2):
                g2_mb2[:, 2 * j + mo] = m * bv[mo * 128:(mo + 1) * 128]
    d["g2_m"], d["g2_mb2"], d["g2_bn"] = g2_m, g2_mb2, g2_bn

    d["fcb"] = np.tile(inputs["fc_b"].astype(f).reshape(1, 1000), (n, 1))
    return d


_program_cache = {}
TRACE = False   # set by test harness to capture an NTFF profile
LAST = None     # BassKernelResults of the last run


def kernel(**inputs):
    global LAST
    B = inputs["inputs"].shape[0]
    assert B % NCORES == 0
    n = B // NCORES
    npair = n // 2
    policy = (np.asarray(inputs["probs"]) >= 0.5)
    A = optimize_assignment(policy, NCORES, n)

    polA = policy[A]  # [cores, n, 15]
    ku0 = tuple(tuple(bool(polA[:, 2 * j:2 * j + 2, b].any())
                      for b in range(5)) for j in range(npair))
    ka0 = tuple(tuple(bool(polA[:, 2 * j:2 * j + 2, b].all())
                      for b in range(5)) for j in range(npair))
    ku1 = tuple(tuple(bool(polA[:, s, 5 + b].any()) for b in range(5))
                for s in range(n))
    ka1 = tuple(tuple(bool(polA[:, s, 5 + b].all()) for b in range(5))
                for s in range(n))
    ku2 = tuple(tuple(bool(polA[:, s, 10 + b].any()) for b in range(5))
                for s in range(n))
    ka2 = tuple(tuple(bool(polA[:, s, 10 + b].all()) for b in range(5))
                for s in range(n))

    key = (n, ku0, ka0, ku1, ka1, ku2, ka2)
    if key not in _program_cache:
        _program_cache.clear()
        _program_cache[key] = build_program(n, ku0, ka0, ku1, ka1, ku2, ka2)
    nc = _program_cache[key]
    shared = _prep_shared(inputs)
    polf = policy.astype(np.float32)
    in_maps = [_prep_core(inputs, shared, polf, A, c, n) for c in range(NCORES)]
    res = run_bass_kernel_spmd(nc, in_maps, core_ids=list(range(NCORES)),
                               trace=TRACE)
    LAST = res
    globals()["LAST_NC"] = nc
    globals()["LAST_IN_MAPS"] = in_maps
    out = np.zeros((B, 1000), np.float32)
    for c in range(NCORES):
        out[A[c]] = res.results[c]["y"]
    return out


# revision 33
# speedup vs baseline: 1.1270x; 1.1270x over previous
"""BlockDrop ResNet kernel for Trainium2 (8 NeuronCores, data parallel).

Network: seed conv3x3 (3->64) + ReLU, then 3 groups of 5 residual blocks
(g0: 64ch @64x64, g1: 128ch @32x32, g2: 256ch @16x16; first block of each
group strided with 1x1-downsample residual), each block gated per-sample by
policy = (probs >= 0.5); then global average pool + FC to 1000 classes.

Key optimizations over the direct per-sample formulation:
- g0 (and the seed conv) pack TWO samples per matmul with block-diagonal
  weights: sample A's 64 channels live on partitions 0:64, sample B's on
  64:128, so the K=64 convs still use the full 128-wide PE array.
- The program is specialized to the actual policy: a block's convs are
  emitted only if ANY of the samples assigned to that slot (across all 8
  SPMD cores; 16 samples for paired g0 slots) keeps the block.  A host-side
  local-search assigner clusters samples with similar policies onto the
  same slot to maximize skipped blocks.  Per-sample correctness is kept by
  per-partition mask columns in the gated blend.
- Slots where ALL samples keep a block use a short blend
  (out = relu(conv2 + res + b2): 1 DVE + 1 Act op instead of 3 DVE + 1 Act).
Convs are computed as 9 accumulating matmuls over (dy,dx) shifts with
channels on the partition (contraction) dim, activations stored padded
([C, (H+2)*(W+2)]) in SBUF as float32r.
"""

import numpy as np
from contextlib import ExitStack

import concourse.bass as bass
import concourse.tile as tile
from concourse import mybir
from concourse.bass_utils import run_bass_kernel_spmd
from concourse.vector_clock import ScopedClock

F32 = mybir.dt.float32
F32R = mybir.dt.float32r
U32 = mybir.dt.uint32
AF = mybir.ActivationFunctionType
NCORES = 8
SHIFTS = [(dy, dx) for dy in range(3) for dx in range(3)]
ZERO_EACH = False   # debug: memset every padded tile allocation (CoreSim)

# ---------------------------------------------------------------------------
# Workarounds for this walrus build: TPB instructions may carry at most one
# embedded sem wait. Tile attaches multi-waits both to body instructions and
# to the kernel-tail drain; hoist the excess onto same-engine NOPs.
MAX_WAITS = 1
_wsplit_counter = [0]


def _split_excess_waits(nc, max_waits=MAX_WAITS):
    n_split = 0
    for f in nc.m.functions:
        for bb in f.blocks:
            changed = False
            new = []
            for ins in bb.instructions:
                si = ins.sync_info
                if si is not None and len(si.on_wait) > max_waits:
                    waits = list(si.on_wait)
                    keep = waits[:max_waits]
                    extra = waits[max_waits:]
                    for i in range(0, len(extra), max_waits):
                        _wsplit_counter[0] += 1
                        nop = mybir.InstNoOp(
                            name=f"I-wsplit-{_wsplit_counter[0]}", ins=[], outs=[])
                        nop.engine = ins.engine
                        nop.sync_info = mybir.SyncInfo(
                            on_wait=extra[i:i + max_waits], on_update=[])
                        new.append(nop)
                        n_split += 1
                    ins.sync_info = mybir.SyncInfo(
                        on_wait=keep, on_update=list(si.on_update))
                    changed = True
                new.append(ins)
            if changed:
                bb.instructions = new
    return n_split


def _patched_drain_and_barrier(self, tick_clock, wait_clock):
    nc = self.nc
    probe = nc.sync.nop(nofuse=True, hint="tail_drain_waits")
    wait_clock.add_sem_waits(
        probe.ins, ScopedClock({None: tick_clock.global_clock}))
    si = probe.ins.sync_info
    waits = list(si.on_wait) if si is not None else []
    probe.ins.sync_info = mybir.SyncInfo(on_wait=waits[:1], on_update=[])
    for i in range(1, len(waits)):
        n2 = nc.sync.nop(nofuse=True, hint=f"tail_drain_waits_{i}")
        n2.ins.sync_info = mybir.SyncInfo(on_wait=waits[i:i + 1], on_update=[])
    nc.sync.drain()
    nc.all_engine_barrier()
    assert self.sems is not None
    popped = nc._tile_sem_poison_stack.pop()
    assert popped is self._sem_poison
    nc.clear_and_free_semaphores(list(self.sems.allocated().values()))
    nc.all_engine_barrier()


tile.TileContext._drain_and_barrier = _patched_drain_and_barrier


# ---------------------------------------------------------------------------
def r3(ap, c):
    return ap.rearrange("p (r c) -> p r c", c=c)


def build_program(n, ku0, ka0, ku1, ka1, ku2, ka2):
    """Build the SPMD program for n samples per core (n even).

    ku0[j][b]: pair-slot j computes g0 block b (any of its 16 samples keeps).
    ka0[j][b]: ALL of its samples keep (short blend).  ku1/ka1, ku2/ka2: the
    same per slot s (8 samples) for g1/g2.
    """
    assert n % 2 == 0
    npair = n // 2
    nc = bass.Bass()

    P = {}

    def param(name, shape, dt=F32R, out=False):
        P[name] = nc.declare_dram_parameter(name, list(shape), dt, isOutput=out)
        return P[name]

    param("x", [n, 3, 64, 64])
    param("seed_wBD", [64, 128])
    param("seed_b", [128, 1], F32)
    param("g0_dsBD", [128, 128])
    param("g0_w1BD", [128, 1152])
    param("g0_w2BD", [128, 1152])
    param("g0_rw1BD", [4, 128, 1152])
    param("g0_rw2BD", [4, 128, 1152])
    param("g0_dsb", [128, 1], F32)
    param("g0_b1c", [128, 5], F32)
    param("g1_ds_wT", [128, 128])
    param("g1_w1T", [128, 1152])
    param("g1_w2T", [128, 1152])
    param("g1_rw1T", [4, 128, 1152])
    param("g1_rw2T", [4, 128, 1152])
    param("g1_dsb", [128, 1], F32)
    param("g1_b1c", [128, 5], F32)
    param("g2_ds_wT", [128, 256])
    param("g2_w1T", [128, 2304])
    param("g2_w2T", [128, 4608])
    param("g2_rw1T", [4, 128, 4608])
    param("g2_rw2T", [4, 128, 4608])
    param("g2_dsb", [128, 2], F32)
    param("g2_b1c", [128, 10], F32)
    param("g0_m", [128, 5 * npair], F32)
    param("g0_mb2", [128, 5 * npair], F32)
    param("g0_bn", [128, 5 * npair], F32)
    for g in (1, 2):
        param(f"g{g}_m", [128, 5 * n], F32)
        param(f"g{g}_bn", [128, 5 * n], F32)
    param("g1_mb2", [128, 5 * n], F32)
    param("g2_mb2", [128, 2 * 5 * n], F32)
    param("fc_wT", [128, 2000])
    param("fcb", [n, 1000], F32)
    param("y", [n, 1000], F32, out=True)

    stage1 = nc.dram_tensor("stage1", [n, 128, 34 * 34], F32R)

    def pad_tile(pool, shape, tag):
        t = pool.tile(shape, F32R, tag=tag, name=tag)
        if ZERO_EACH:
            nc.gpsimd.memset(t[:].bitcast(U32), 0)
        return t

    with tile.TileContext(nc) as tc, ExitStack() as ctx:
        cpool = ctx.enter_context(tc.tile_pool(name="const", bufs=1))
        pp = ctx.enter_context(tc.tile_pool(name="psum", bufs=6, space="PSUM"))
        ppd = ctx.enter_context(tc.tile_pool(name="psumd", bufs=2, space="PSUM"))

        def ctile(pname, shape=None, dt=F32):
            t = cpool.tile(shape or [s for s in P[pname].shape], dt,
                           tag=pname, name=pname)
            nc.sync.dma_start(t[:], P[pname][:])
            return t

        # constants resident for the whole program
        seed_w = cpool.tile([64, 128], F32R, tag="seed_wBD", name="seed_wBD")
        nc.sync.dma_start(seed_w[:], P["seed_wBD"][:])
        seed_b = ctile("seed_b")
        pooled = [cpool.tile([128, n], F32R, tag=f"pooled{k}", name=f"pooled{k}")
                  for k in range(2)]

        # ------------------------------------------------------------------
        # Stage 1: per pair, seed conv + g0 (5 paired blocks) + g1 per sample
        # (b0 stride-2 from the g0 output, then 4 identity blocks); final g1
        # activation (padded, f32r) staged to DRAM.  Emission interleaves
        # pair j's g1 with pair j+1's g0 (and g1's two samples with each
        # other) so the PE never drains at block boundaries.
        with ExitStack() as s1:
            wres = s1.enter_context(tc.tile_pool(name="wres", bufs=1))
            wstr = s1.enter_context(tc.tile_pool(name="wstr", bufs=3))
            patchp = s1.enter_context(tc.tile_pool(name="patch", bufs=2))
            g0genp = s1.enter_context(tc.tile_pool(name="g0gen", bufs=3))
            g0hp = s1.enter_context(tc.tile_pool(name="g0h", bufs=1))
            g1genp = s1.enter_context(tc.tile_pool(name="g1gen", bufs=4))
            g1hp = s1.enter_context(tc.tile_pool(name="g1h", bufs=2))
            tmpp = s1.enter_context(tc.tile_pool(name="tmp", bufs=2))
            resp = s1.enter_context(tc.tile_pool(name="res", bufs=1))
            res1p = s1.enter_context(tc.tile_pool(name="res1", bufs=2))

            xflat = P["x"].rearrange("n c h w -> (n c) h w")
            patch_views = {}

            def emit_patch(j):
                pt = pad_tile(patchp, [64, 64 * 64], "patch")
                vpt = r3(pt[:], 64)
                engs = [nc.sync, nc.scalar] if j == 0 else [nc.sync, nc.gpsimd]
                for si, (dy, dx) in enumerate(SHIFTS):
                    rr0, rr1 = max(0, 1 - dy), min(64, 65 - dy)
                    cc0, cc1 = max(0, 1 - dx), min(64, 65 - dx)
                    engs[si % 2].dma_start(
                        vpt[6 * si:6 * si + 6, rr0:rr1, cc0:cc1],
                        xflat[6 * j:6 * j + 6, rr0 + dy - 1:rr1 + dy - 1,
                              cc0 + dx - 1:cc1 + dx - 1])
                patch_views[j] = vpt

            def wt_res(pname):
                t = wres.tile([s for s in P[pname].shape], F32R,
                              tag=pname, name=pname)
                nc.sync.dma_start(t[:], P[pname][:])
                return t

            latec = {}

            def emit_late_consts():
                for nm in ("g0_m", "g0_mb2", "g0_bn", "g1_m", "g1_mb2",
                           "g1_bn", "g1_b1c", "g1_dsb", "g2_m", "g2_mb2",
                           "g2_bn", "g2_b1c", "g2_dsb"):
                    latec[nm] = ctile(nm)

            def wt_str(pname, j, tag, eng=None):
                t = wstr.tile([128, 1152], F32R, tag=tag, name=f"{tag}{j}")
                src = P[pname] if j is None else P[pname][j]
                (eng or nc.sync).dma_start(t[:], src[:])
                return t

            # pre-zero the rotating padded buffers once (borders stay zero:
            # every later generation writes only the interior cells)
            def prezero(pool, bufs, shape, tag):
                for _ in range(bufs):
                    t = pool.tile(shape, F32R, tag=tag, name=tag)
                    nc.gpsimd.memset(t[:].bitcast(U32), 0)

            prezero(patchp, 2, [64, 64 * 64], "patch")
            emit_patch(0)
            g0_ds = wt_res("g0_dsBD")
            g0_dsb = ctile("g0_dsb")
            g0_b1c = ctile("g0_b1c")
            preload = {}
            preload[("w1", 0)] = wt_str("g0_w1BD", None, "rw1", nc.scalar)
            preload[("w2", 0)] = wt_str("g0_w2BD", None, "rw2", nc.scalar)
            g1_ds = wt_res("g1_ds_wT")
            g1_w1 = wt_res("g1_w1T")
            g1_w2 = wt_res("g1_w2T")
            prezero(g0genp, 3, [128, 66 * 66], "g0x")
            prezero(g0hp, 1, [128, 66 * 66], "g0h")
            prezero(g1genp, 4, [128, 34 * 34], "g1x")
            prezero(g1hp, 2, [128, 34 * 34], "g1h")

            def blend(ps2, res_ap, out_ap, mcol, mb2col, bncol, C, allkeep):
                """out = Relu(m*(ps2+res) + m*b2) + (1-m)*res (views [p,R,C]).
                allkeep: every gated sample has m=1 -> out = Relu(t + b2)."""
                R = ps2.shape[1]
                t = r3(tmpp.tile([128, R * C], F32, tag="t", name="t")[:], C)
                nc.vector.tensor_add(t, ps2, res_ap)
                if allkeep:
                    nc.scalar.activation(out_ap, t, AF.Relu, bias=mb2col,
                                         scale=1.0)
                    return
                rm = r3(tmpp.tile([128, R * C], F32, tag="rm", name="rm")[:], C)
                nc.scalar.activation(rm, t, AF.Relu, bias=mb2col, scale=mcol)
                rb = r3(tmpp.tile([128, R * C], F32, tag="rb", name="rb")[:], C)
                nc.scalar.mul(rb, res_ap, bncol)
                nc.vector.tensor_add(out_ap, rm, rb)

            # ---- unit builders.  A unit is (est_pe_rows, closure); closures
            # allocate tiles and emit instructions when called, so emission
            # order == engine queue order.  State flows through per-pair dicts.
            def g0_units(j):
                st = {}
                units = []

                def u_seed():
                    vpt = patch_views[j]
                    xg = pad_tile(g0genp, [128, 66 * 66], "g0x")
                    vg = r3(xg[:], 66)
                    for c in range(8):
                        ps = pp.tile([128, 512], F32, tag="ps", name="ps")
                        r0 = c * 8
                        nc.tensor.matmul(ps[:], seed_w[0:64, :],
                                         vpt[0:64, r0:r0 + 8, 0:64])
                        ov = vg[:, 1 + r0:1 + r0 + 8, 1:65]
                        if c % 2 == 0:
                            nc.scalar.activation(ov, r3(ps[:], 64), AF.Relu,
                                                 bias=seed_b[:, 0:1], scale=1.0)
                        else:
                            nc.vector.tensor_scalar(
                                ov, r3(ps[:], 64), seed_b[:, 0:1], 0.0,
                                mybir.AluOpType.add, mybir.AluOpType.max)
                    st["vg"] = vg

                units.append((4096, u_seed))
                if j + 1 < npair:
                    units.append((0, lambda: emit_patch(j + 1)))

                def u_ds(keep):
                    def fn():
                        vg = st["vg"]
                        if keep:
                            restile = resp.tile([128, 4096], F32, tag="g0res",
                                                name="g0res")
                            st["res"] = restile
                        else:
                            xg2 = pad_tile(g0genp, [128, 66 * 66], "g0x")
                            vg2 = r3(xg2[:], 66)
                        for c in range(8):
                            psd = pp.tile([128, 512], F32, tag="ps",
                                          name="ps")
                            nc.tensor.matmul(psd[:], g0_ds[:],
                                             vg[:, c * 8 + 1:c * 8 + 9, 1:65])
                            if keep:
                                ov, iv = restile[:, c * 512:(c + 1) * 512], psd[:]
                            else:
                                ov, iv = (vg2[:, c * 8 + 1:c * 8 + 9, 1:65],
                                          r3(psd[:], 64))
                            if c % 2 == 0:
                                nc.scalar.activation(
                                    ov, iv, AF.Identity, bias=g0_dsb[:, 0:1],
                                    scale=1.0)
                            else:
                                nc.vector.tensor_scalar_add(ov, iv,
                                                            g0_dsb[:, 0:1])
                        if not keep:
                            st["vg"] = vg2

                    return fn

                def u_conv1(b):
                    def fn():
                        if b == 0:
                            w1 = preload.pop(("w1", j), None) or \
                                wt_str("g0_w1BD", None, "rw1")
                        else:
                            w1 = wt_str("g0_rw1BD", b - 1, "rw1")
                        vg = st["vg"]
                        h = pad_tile(g0hp, [128, 66 * 66], "g0h")
                        vh = r3(h[:], 66)
                        b1col = g0_b1c[:, b:b + 1]
                        for c in range(8):
                            ps1 = pp.tile([128, 512], F32, tag="ps", name="ps")
                            for si, (dy, dx) in enumerate(SHIFTS):
                                nc.tensor.matmul(
                                    ps1[:], w1[:, si * 128:(si + 1) * 128],
                                    vg[:, c * 8 + dy:c * 8 + dy + 8, dx:dx + 64],
                                    start=(si == 0), stop=(si == 8))
                            nc.scalar.activation(
                                vh[:, 1 + c * 8:1 + c * 8 + 8, 1:65],
                                r3(ps1[:], 64), AF.Relu, bias=b1col, scale=1.0)
                        st["vh"] = vh

                    return fn

                def u_conv2(b):
                    def fn():
                        if b == 0:
                            w2 = preload.pop(("w2", j), None) or \
                                wt_str("g0_w2BD", None, "rw2")
                        else:
                            w2 = wt_str("g0_rw2BD", b - 1, "rw2")
                        vg, vh = st["vg"], st["vh"]
                        col = b * npair + j
                        mcol = latec["g0_m"][:, col:col + 1]
                        mb2col = latec["g0_mb2"][:, col:col + 1]
                        bncol = latec["g0_bn"][:, col:col + 1]
                        xg2 = pad_tile(g0genp, [128, 66 * 66], "g0x")
                        vg2 = r3(xg2[:], 66)
                        for c in range(8):
                            ps2 = pp.tile([128, 512], F32, tag="ps", name="ps")
                            for si, (dy, dx) in enumerate(SHIFTS):
                                nc.tensor.matmul(
                                    ps2[:], w2[:, si * 128:(si + 1) * 128],
                                    vh[:, c * 8 + dy:c * 8 + dy + 8, dx:dx + 64],
                                    start=(si == 0), stop=(si == 8))
                            if b == 0:
                                res_ap = r3(st["res"][:, c * 512:(c + 1) * 512],
                                            64)
                            else:
                                res_ap = vg[:, 1 + c * 8:1 + c * 8 + 8, 1:65]
                            blend(r3(ps2[:], 64), res_ap,
                                  vg2[:, 1 + c * 8:1 + c * 8 + 8, 1:65],
                                  mcol, mb2col, bncol, 64, ka0[j][b])
                        st["vg"] = vg2

                    return fn

                units.append((4096, u_ds(ku0[j][0])))
                first_conv = True
                for b in range(5):
                    if ku0[j][b]:
                        units.append((36864, u_conv1(b)))
                        if j == 0 and first_conv:
                            units.append((0, emit_late_consts))
                            first_conv = False
                        units.append((36864, u_conv2(b)))
                if j == 0 and first_conv:
                    units.append((0, emit_late_consts))
                st_holder[j] = st
                return units

            def g1_units_sample(j, half, s):
                st = {}
                units = []
                keep0 = ku1[s][0]

                def u_b0c1():
                    vg = st_holder[j]["vg"]
                    x1 = pad_tile(g1genp, [128, 34 * 34], "g1x")
                    vx1 = r3(x1[:], 34)
                    st["vx1"] = vx1
                    st["x1"] = x1
                    if keep0:
                        res1 = res1p.tile([128, 1024], F32, tag="g1res",
                                          name="g1res")
                        st["res1"] = res1
                        h1 = pad_tile(g1hp, [128, 34 * 34], "g1h")
                        vh1 = r3(h1[:], 34)
                        st["vh1"] = vh1
                    for c in range(2):
                        psd = ppd.tile([128, 512], F32, tag="psds", name="psds")
                        nc.tensor.matmul(
                            psd[:], g1_ds[half:half + 64, :],
                            vg[half:half + 64,
                               1 + 32 * c:1 + 32 * c + 32:2, 1:65:2])
                        if keep0:
                            nc.scalar.activation(
                                res1[:, c * 512:(c + 1) * 512], psd[:],
                                AF.Identity, bias=latec["g1_dsb"][:, 0:1], scale=1.0)
                            ps1 = pp.tile([128, 512], F32, tag="ps", name="ps")
                            for si, (dy, dx) in enumerate(SHIFTS):
                                nc.tensor.matmul(
                                    ps1[:],
                                    g1_w1[half:half + 64,
                                          si * 128:(si + 1) * 128],
                                    vg[half:half + 64,
                                       dy + 32 * c:dy + 32 * c + 32:2,
                                       dx:dx + 64:2],
                                    start=(si == 0), stop=(si == 8))
                            nc.scalar.activation(
                                vh1[:, 1 + 16 * c:1 + 16 * c + 16, 1:33],
                                r3(ps1[:], 32),
                                AF.Relu, bias=latec["g1_b1c"][:, 0:1], scale=1.0)
                        else:
                            nc.scalar.activation(
                                vx1[:, 1 + 16 * c:1 + 16 * c + 16, 1:33],
                                r3(psd[:], 32),
                                AF.Identity, bias=latec["g1_dsb"][:, 0:1], scale=1.0)

                def u_b0c2():
                    mcol = latec["g1_m"][:, s:s + 1]
                    mb2col = latec["g1_mb2"][:, s:s + 1]
                    bncol = latec["g1_bn"][:, s:s + 1]
                    vx1, vh1 = st["vx1"], st["vh1"]
                    res1 = st["res1"]
                    for c in range(2):
                        ps2 = pp.tile([128, 512], F32, tag="ps", name="ps")
                        for si, (dy, dx) in enumerate(SHIFTS):
                            nc.tensor.matmul(
                                ps2[:], g1_w2[:, si * 128:(si + 1) * 128],
                                vh1[:, dy + 16 * c:dy + 16 * c + 16, dx:dx + 32],
                                start=(si == 0), stop=(si == 8))
                        blend(r3(ps2[:], 32),
                              r3(res1[:, c * 512:(c + 1) * 512], 32),
                              vx1[:, 1 + 16 * c:1 + 16 * c + 16, 1:33],
                              mcol, mb2col, bncol, 32, ka1[s][0])

                units.append((10240 if keep0 else 1024, u_b0c1))
                if keep0:
                    units.append((9216, u_b0c2))

                def u_c1(b):
                    def fn():
                        w1 = wt_str("g1_rw1T", b - 1, "rw1")
                        vx1 = st["vx1"]
                        h1 = pad_tile(g1hp, [128, 34 * 34], "g1h")
                        vh1 = r3(h1[:], 34)
                        st["vh1"] = vh1
                        for c in range(2):
                            ps1 = pp.tile([128, 512], F32, tag="ps", name="ps")
                            for si, (dy, dx) in enumerate(SHIFTS):
                                nc.tensor.matmul(
                                    ps1[:], w1[:, si * 128:(si + 1) * 128],
                                    vx1[:, dy + 16 * c:dy + 16 * c + 16,
                                        dx:dx + 32],
                                    start=(si == 0), stop=(si == 8))
                            nc.scalar.activation(
                                vh1[:, 1 + 16 * c:1 + 16 * c + 16, 1:33],
                                r3(ps1[:], 32), AF.Relu,
                                bias=latec["g1_b1c"][:, b:b + 1], scale=1.0)

                    return fn

                def u_c2(b):
                    def fn():
                        w2 = wt_str("g1_rw2T", b - 1, "rw2")
                        mi = b * n + s
                        mcol = latec["g1_m"][:, mi:mi + 1]
                        mb2col = latec["g1_mb2"][:, mi:mi + 1]
                        bncol = latec["g1_bn"][:, mi:mi + 1]
                        vx1, vh1 = st["vx1"], st["vh1"]
                        x1n = pad_tile(g1genp, [128, 34 * 34], "g1x")
                        vx1n = r3(x1n[:], 34)
                        for c in range(2):
                            ps2 = pp.tile([128, 512], F32, tag="ps", name="ps")
                            for si, (dy, dx) in enumerate(SHIFTS):
                                nc.tensor.matmul(
                                    ps2[:], w2[:, si * 128:(si + 1) * 128],
                                    vh1[:, dy + 16 * c:dy + 16 * c + 16,
                                        dx:dx + 32],
                                    start=(si == 0), stop=(si == 8))
                            blend(r3(ps2[:], 32),
                                  vx1[:, 1 + 16 * c:1 + 16 * c + 16, 1:33],
                                  vx1n[:, 1 + 16 * c:1 + 16 * c + 16, 1:33],
                                  mcol, mb2col, bncol, 32, ka1[s][b])
                        st["vx1"], st["x1"] = vx1n, x1n

                    return fn

                for b in range(1, 5):
                    if ku1[s][b]:
                        units.append((9216, u_c1(b)))
                        units.append((9216, u_c2(b)))
                units.append((0, lambda: nc.sync.dma_start(stage1[s],
                                                           st["x1"][:])))
                return units

            def merge_units(a, b):
                out = []
                ia = ib = 0
                ta = tb = 0
                while ia < len(a) or ib < len(b):
                    if ib >= len(b) or (ia < len(a) and ta <= tb):
                        est, fn = a[ia]
                        ia += 1
                        ta += est
                        out.append((est, fn))
                    else:
                        est, fn = b[ib]
                        ib += 1
                        tb += est
                        out.append((est, fn))
                return out

            st_holder = {}
            prev_g1 = []
            for j in range(npair):
                for est, fn in merge_units(prev_g1, g0_units(j)):
                    fn()
                prev_g1 = merge_units(
                    g1_units_sample(j, 0, 2 * j),
                    g1_units_sample(j, 64, 2 * j + 1))
            for est, fn in prev_g1:
                fn()

        # ------------------------------------------------------------------
        # Stage 2: g2, block-outer over all samples; then pool + fc.
        with ExitStack() as s2:
            w2p = s2.enter_context(tc.tile_pool(name="wg2", bufs=2))
            g2inp = s2.enter_context(tc.tile_pool(name="g2in", bufs=3))
            g2genp = s2.enter_context(tc.tile_pool(name="g2gen", bufs=n + 4))
            g2hp = s2.enter_context(tc.tile_pool(name="g2h", bufs=3))
            tmp2p = s2.enter_context(tc.tile_pool(name="tmp2", bufs=3))

            for _ in range(n + 4):
                t = g2genp.tile([128, 2 * 324], F32R, tag="g2x", name="g2x")
                nc.gpsimd.memset(t[:].bitcast(U32), 0)
            for _ in range(3):
                t = g2hp.tile([128, 2 * 324], F32R, tag="g2h", name="g2h")
                nc.gpsimd.memset(t[:].bitcast(U32), 0)

            def g2_conv(ps_pair, w, views, nshift=9):
                nki = len(views)
                for ki in range(nki):
                    for si in range(nshift):
                        for mo in range(2):
                            col = ki * nshift * 256 + si * 256 + mo * 128
                            nc.tensor.matmul(
                                ps_pair[mo][:], w[:, col:col + 128],
                                views[ki][si],
                                start=(ki == 0 and si == 0),
                                stop=(ki == nki - 1 and si == nshift - 1))

            cur = [None] * n  # current g2 activation tile per sample slot
            lastb = [max([0] + [b for b in range(5) if ku2[s][b]])
                     for s in range(n)]

            def emit_pool(s):
                vxg = [r3(cur[s][:, ki * 324:(ki + 1) * 324], 18)
                       for ki in range(2)]
                with nc.allow_low_precision(reason="pooled stored f32r"):
                    for ki in range(2):
                        nc.vector.reduce_sum(pooled[ki][:, s:s + 1],
                                             vxg[ki][:, 1:17, 1:17],
                                             axis=mybir.AxisListType.XY)

            for b in range(5):
                any_b = any(ku2[s][b] for s in range(n)) or b == 0
                if any_b and b == 0:
                    dsT = w2p.tile([128, 256], F32R, tag="g2ds", name="g2ds")
                    nc.scalar.dma_start(dsT[:], P["g2_ds_wT"][:])
                    w1t = w2p.tile([128, 2304], F32R, tag="g2w1", name="g2w1")
                    nc.scalar.dma_start(w1t[:], P["g2_w1T"][:])
                    w2t = w2p.tile([128, 4608], F32R, tag="g2w2", name="g2w2")
                    nc.scalar.dma_start(w2t[:], P["g2_w2T"][:])
                elif any_b and b > 0:
                    w1t = w2p.tile([128, 4608], F32R, tag="g2w1", name="g2w1")
                    nc.sync.dma_start(w1t[:], P["g2_rw1T"][b - 1])
                    w2t = w2p.tile([128, 4608], F32R, tag="g2w2", name="g2w2")
                    nc.sync.dma_start(w2t[:], P["g2_rw2T"][b - 1])

                def part1(b, s, w1t, dsT):
                    """ds (b0) + conv1 + h2 acts; returns state for part2, or
                    None if fully handled (dropped b0)."""
                    if b == 0:
                        x1t = g2inp.tile([128, 34 * 34], F32R, tag="g2in",
                                         name="g2in")
                        (nc.scalar if s % 2 == 0 else nc.sync).dma_start(
                            x1t[:], stage1[s])
                        vin = r3(x1t[:], 34)
                        if not ku2[s][0]:
                            nxt = pad_tile(g2genp, [128, 2 * 324], "g2x")
                            vnxt = [r3(nxt[:, ki * 324:(ki + 1) * 324], 18)
                                    for ki in range(2)]
                            for mo in range(2):
                                psd = ppd.tile([128, 256], F32, tag="psds",
                                               name="psds")
                                nc.tensor.matmul(
                                    psd[:], dsT[:, mo * 128:(mo + 1) * 128],
                                    vin[:, 1:33:2, 1:33:2])
                                nc.scalar.activation(
                                    vnxt[mo][:, 1:17, 1:17], r3(psd[:], 16),
                                    AF.Identity,
                                    bias=latec["g2_dsb"][:, mo:mo + 1],
                                    scale=1.0)
                            cur[s] = nxt
                            if lastb[s] == 0:
                                emit_pool(s)
                            return None
                        restmp = tmp2p.tile([128, 512], F32, tag="res2",
                                            name="res2")
                        for mo in range(2):
                            psd = ppd.tile([128, 256], F32, tag="psds",
                                           name="psds")
                            nc.tensor.matmul(
                                psd[:], dsT[:, mo * 128:(mo + 1) * 128],
                                vin[:, 1:33:2, 1:33:2])
                            nc.scalar.activation(
                                restmp[:, mo * 256:(mo + 1) * 256], psd[:],
                                AF.Identity, bias=latec["g2_dsb"][:, mo:mo + 1],
                                scale=1.0)
                        res_aps = [r3(restmp[:, mo * 256:(mo + 1) * 256], 16)
                                   for mo in range(2)]
                        w1_views = [[vin[:, dy:dy + 32:2, dx:dx + 32:2]
                                     for (dy, dx) in SHIFTS]]
                    else:
                        xg = cur[s]
                        vxg = [r3(xg[:, ki * 324:(ki + 1) * 324], 18)
                               for ki in range(2)]
                        res_aps = [vxg[mo][:, 1:17, 1:17] for mo in range(2)]
                        w1_views = [[vxg[ki][:, dy:dy + 16, dx:dx + 16]
                                     for (dy, dx) in SHIFTS] for ki in range(2)]
                    h2 = pad_tile(g2hp, [128, 2 * 324], "g2h")
                    vh2 = [r3(h2[:, ki * 324:(ki + 1) * 324], 18)
                           for ki in range(2)]
                    ps1p = [pp.tile([128, 256], F32, tag="ps", name="ps")
                            for _ in range(2)]
                    g2_conv(ps1p, w1t, w1_views)
                    for mo in range(2):
                        nc.scalar.activation(
                            vh2[mo][:, 1:17, 1:17], r3(ps1p[mo][:], 16),
                            AF.Relu,
                            bias=latec["g2_b1c"][:, 2 * b + mo:2 * b + mo + 1],
                            scale=1.0)
                    return {"vh2": vh2, "res_aps": res_aps}

                def part2(b, s, st, w2t):
                    mcol = latec["g2_m"][:, b * n + s:b * n + s + 1]
                    bncol = latec["g2_bn"][:, b * n + s:b * n + s + 1]
                    mb2c = [latec["g2_mb2"][:, (b * n + s) * 2 + mo:
                                            (b * n + s) * 2 + mo + 1]
                            for mo in range(2)]
                    vh2, res_aps = st["vh2"], st["res_aps"]
                    ps2p = [pp.tile([128, 256], F32, tag="ps", name="ps")
                            for _ in range(2)]
                    g2_conv(ps2p, w2t,
                            [[vh2[ki][:, dy:dy + 16, dx:dx + 16]
                              for (dy, dx) in SHIFTS] for ki in range(2)])
                    nxt = pad_tile(g2genp, [128, 2 * 324], "g2x")
                    vnxt = [r3(nxt[:, ki * 324:(ki + 1) * 324], 18)
                            for ki in range(2)]
                    for mo in range(2):
                        t2 = r3(tmp2p.tile([128, 256], F32, tag="t2",
                                           name="t2")[:], 16)
                        nc.vector.tensor_add(t2, r3(ps2p[mo][:], 16),
                                             res_aps[mo])
                        if ka2[s][b]:
                            nc.scalar.activation(vnxt[mo][:, 1:17, 1:17], t2,
                                                 AF.Relu, bias=mb2c[mo],
                                                 scale=1.0)
                            continue
                        rm = r3(tmp2p.tile([128, 256], F32, tag="rm2",
                                           name="rm2")[:], 16)
                        nc.scalar.activation(rm, t2, AF.Relu,
                                             bias=mb2c[mo], scale=mcol)
                        rb = r3(tmp2p.tile([128, 256], F32, tag="rb2",
                                           name="rb2")[:], 16)
                        nc.vector.tensor_scalar_mul(rb, res_aps[mo], bncol)
                        nc.vector.tensor_add(vnxt[mo][:, 1:17, 1:17], rm, rb)
                    cur[s] = nxt
                    if lastb[s] == b:
                        emit_pool(s)

                prev = None
                for s in range(n):
                    if not ku2[s][b] and b > 0:
                        continue
                    st = part1(b, s, w1t, dsT if b == 0 else None)
                    if st is None:
                        continue
                    if prev is not None:
                        part2(b, prev[0], prev[1], w2t)
                    prev = (s, st)
                if prev is not None:
                    part2(b, prev[0], prev[1], w2t)

            # fc: y[s, k] = pooled[:, s] . fc_wT[:, k]
            fcw = w2p.tile([128, 2000], F32R, tag="fcw", name="fcw")
            nc.scalar.dma_start(fcw[:], P["fc_wT"][:])
            fcbt = w2p.tile([n, 1000], F32, tag="fcb", name="fcb")
            nc.scalar.dma_start(fcbt[:], P["fcb"][:])
            outsb = w2p.tile([n, 1000], F32, tag="outsb", name="outsb")
            for nb, (n0, n1) in enumerate([(0, 512), (512, 1000)]):
                psf = ppd.tile([n, n1 - n0], F32, tag="psds", name="psds")
                for ki in range(2):
                    nc.tensor.matmul(psf[:], pooled[ki][:, 0:n],
                                     fcw[:, ki * 1000 + n0:ki * 1000 + n1],
                                     start=(ki == 0), stop=(ki == 1))
                nc.vector.tensor_add(outsb[:, n0:n1], psf[:], fcbt[:, n0:n1])
            nc.sync.dma_start(P["y"][:], outsb[:])

    _split_excess_waits(nc)
    return nc


# ---------------------------------------------------------------------------
# Host-side: sample-to-slot assignment optimization.  Minimizes the SPMD
# union-keep compute: a g0 block is computed for a pair-slot if any of its 16
# samples (2 slots x 8 cores) keeps it; g1/g2 blocks per slot (8 samples).
W_G0, W_G1, W_G2B0, W_G2 = 73728, 18432, 13824, 18432


def optimize_assignment(pol, ncores, n, iters=400000, seeds=6):
    """Best-of-N restarts of a swap hill-climb."""
    best = None
    best_cost = None
    for seed in range(seeds):
        A, c = _optimize_assignment_once(pol, ncores, n, iters, seed)
        if best_cost is None or c < best_cost:
            best, best_cost = A, c
    return best


def _optimize_assignment_once(pol, ncores, n, iters, seed):
    npair = n // 2
    rng = np.random.default_rng(seed)
    key = pol[:, 0:5] @ (2 ** np.arange(5)[::-1]) * 1024 + \
        pol[:, 5:15] @ (2 ** np.arange(10)[::-1])
    order = np.argsort(key, kind="stable")
    A = np.zeros((ncores, n), dtype=np.int64)
    for j in range(npair):
        blk = order[2 * ncores * j:2 * ncores * (j + 1)]
        A[:, 2 * j] = blk[:ncores]
        A[:, 2 * j + 1] = blk[ncores:]

    # bitmask encoding: bits 0-4 g0, 5-9 g1, 10-14 g2 keep flags
    pat = [int(v) for v in (pol.astype(np.uint32) @
                            (1 << np.arange(15, dtype=np.uint32)))]
    pop = [bin(m).count("1") for m in range(32)]
    cost_pair = [W_G0 * pop[m & 31] for m in range(32768)]
    cost_slot = [W_G1 * pop[(m >> 5) & 31] + W_G2 * pop[(m >> 11) & 15]
                 + W_G2B0 * ((m >> 10) & 1) for m in range(32768)]
    cols = [[pat[A[c, s]] for c in range(ncores)] for s in range(n)]

    def slot_u(col):
        u = 0
        for v in col:
            u |= v
        return u

    slot_un = [slot_u(cols[s]) for s in range(n)]
    ri = rng.integers(0, ncores, size=2 * iters)
    rs = rng.integers(0, n, size=2 * iters)
    for it in range(iters):
        c0, c1 = ri[2 * it], ri[2 * it + 1]
        s0, s1 = rs[2 * it], rs[2 * it + 1]
        if s0 == s1:
            continue
        j0, j1 = s0 // 2, s1 // 2
        u00, u01 = slot_un[2 * j0], slot_un[2 * j0 + 1]
        u10, u11 = slot_un[2 * j1], slot_un[2 * j1 + 1]
        old = (cost_slot[slot_un[s0]] + cost_slot[slot_un[s1]]
               + cost_pair[u00 | u01] + (cost_pair[u10 | u11]
                                         if j1 != j0 else 0))
        cols[s0][c0], cols[s1][c1] = cols[s1][c1], cols[s0][c0]
        n0, n1 = slot_u(cols[s0]), slot_u(cols[s1])
        nu = {s0: n0, s1: n1}
        p0 = (nu.get(2 * j0, slot_un[2 * j0])
              | nu.get(2 * j0 + 1, slot_un[2 * j0 + 1]))
        p1 = (nu.get(2 * j1, slot_un[2 * j1])
              | nu.get(2 * j1 + 1, slot_un[2 * j1 + 1]))
        new = (cost_slot[n0] + cost_slot[n1]
               + cost_pair[p0] + (cost_pair[p1] if j1 != j0 else 0))
        if new <= old:
            A[c0, s0], A[c1, s1] = A[c1, s1], A[c0, s0]
            slot_un[s0], slot_un[s1] = n0, n1
        else:
            cols[s0][c0], cols[s1][c1] = cols[s1][c1], cols[s0][c0]
    cost = 0
    for s in range(n):
        cost += cost_slot[slot_un[s]]
    for j in range(npair):
        cost += cost_pair[slot_un[2 * j] | slot_un[2 * j + 1]]
    return A, cost


# ---------------------------------------------------------------------------
def _dup(a):
    return np.concatenate([a, a], axis=0)


def _prep_shared(inputs):
    """Weight/bias tensors shared by all cores, in device layout (f32)."""
    f = np.float32
    d = {}
    sw = inputs["seed_w"].astype(f)  # [64, 3, 3, 3]
    swT = np.transpose(sw, (2, 3, 1, 0)).reshape(27, 64)  # [(dy dx ch), o]
    # patch partition layout: row si*6 + smp*3 + c (smp 0 = sample A -> out
    # cols 0:64, smp 1 = B -> cols 64:128)
    z = np.zeros((64, 128), f)
    for si in range(9):
        for c in range(3):
            z[si * 6 + c, 0:64] = swT[si * 3 + c]
            z[si * 6 + 3 + c, 64:128] = swT[si * 3 + c]
    d["seed_wBD"] = z
    d["seed_b"] = _dup(inputs["seed_b"].astype(f).reshape(64, 1))

    def c3bd(w):  # [64,64,3,3] -> [128, 9*128] block-diag per tap
        t = np.transpose(w.astype(f), (1, 2, 3, 0))  # [I, 3, 3, O]
        bd = np.zeros((128, 9 * 128), f)
        for si in range(9):
            blk = t[:, si // 3, si % 3, :]  # [64, 64]
            bd[0:64, si * 128:si * 128 + 64] = blk
            bd[64:128, si * 128 + 64:si * 128 + 128] = blk
        return bd

    dsT = inputs["g0_ds_w"].astype(f).reshape(64, 64).T
    dsbd = np.zeros((128, 128), f)
    dsbd[0:64, 0:64] = dsT
    dsbd[64:128, 64:128] = dsT
    d["g0_dsBD"] = dsbd
    d["g0_w1BD"] = c3bd(inputs["g0_b0_w1"])
    d["g0_w2BD"] = c3bd(inputs["g0_b0_w2"])
    d["g0_rw1BD"] = np.stack([c3bd(inputs["g0_r_w1"][j]) for j in range(4)])
    d["g0_rw2BD"] = np.stack([c3bd(inputs["g0_r_w2"][j]) for j in range(4)])
    d["g0_dsb"] = _dup(inputs["g0_ds_b"].astype(f).reshape(64, 1))
    d["g0_b1c"] = _dup(np.stack(
        [inputs["g0_b0_b1"].astype(f)] +
        [inputs["g0_r_b1"][j].astype(f) for j in range(4)], axis=1))

    def c3(w, dup):  # [O,I,3,3] -> [I(x2 if dup), 9*O]
        O, I = w.shape[0], w.shape[1]
        t = np.transpose(w.astype(f), (1, 2, 3, 0)).reshape(I, 9 * O)
        return _dup(t) if dup else t

    d["g1_ds_wT"] = _dup(inputs["g1_ds_w"].astype(f).reshape(128, 64).T.copy())
    d["g1_w1T"] = c3(inputs["g1_b0_w1"], True)
    d["g1_w2T"] = c3(inputs["g1_b0_w2"], False)
    d["g1_rw1T"] = np.stack([c3(inputs["g1_r_w1"][j], False) for j in range(4)])
    d["g1_rw2T"] = np.stack([c3(inputs["g1_r_w2"][j], False) for j in range(4)])
    d["g1_dsb"] = inputs["g1_ds_b"].astype(f).reshape(128, 1)
    d["g1_b1c"] = np.stack(
        [inputs["g1_b0_b1"].astype(f)] +
        [inputs["g1_r_b1"][j].astype(f) for j in range(4)], axis=1)

    # g2: cols (ki, s, mo, o)
    def g2c3(w):  # [256, I, 3, 3] -> [128, (ki) 9 2 128] with ki blocks of I
        I = w.shape[1]
        t = np.transpose(w.astype(f), (1, 2, 3, 0))  # [I, 3,3, 256]
        t = t.reshape(I // 128, 128, 9, 2, 128)      # [ki, i, s, mo, o]
        t = np.transpose(t, (1, 0, 2, 3, 4)).reshape(128, -1)
        return t

    d["g2_ds_wT"] = inputs["g2_ds_w"].astype(f).reshape(256, 128).T.reshape(
        128, 2, 128).reshape(128, 256)
    d["g2_w1T"] = g2c3(inputs["g2_b0_w1"])
    d["g2_w2T"] = g2c3(inputs["g2_b0_w2"])
    d["g2_rw1T"] = np.stack([g2c3(inputs["g2_r_w1"][j]) for j in range(4)])
    d["g2_rw2T"] = np.stack([g2c3(inputs["g2_r_w2"][j]) for j in range(4)])
    d["g2_dsb"] = inputs["g2_ds_b"].astype(f).reshape(2, 128).T.copy().reshape(
        128, 2)
    b1s = [inputs["g2_b0_b1"].astype(f)] + \
        [inputs["g2_r_b1"][j].astype(f) for j in range(4)]
    d["g2_b1c"] = np.stack([b[mo * 128:(mo + 1) * 128]
                            for b in b1s for mo in range(2)], axis=1)
    d["fc_wT"] = (inputs["fc_w"].astype(f).T / 256.0).reshape(
        2, 128, 1000).transpose(1, 0, 2).reshape(128, 2000)
    return d


def _prep_core(inputs, shared, policy, A, c, n):
    f = np.float32
    npair = n // 2
    sel = A[c]  # global sample index per slot
    d = dict(shared)
    d["x"] = np.ascontiguousarray(inputs["inputs"][sel].astype(f))
    pol = policy[sel]  # [n, 15]

    b2_g0 = [inputs["g0_b0_b2"]] + [inputs["g0_r_b2"][j] for j in range(4)]
    b2_g1 = [inputs["g1_b0_b2"]] + [inputs["g1_r_b2"][j] for j in range(4)]
    b2_g2 = [inputs["g2_b0_b2"]] + [inputs["g2_r_b2"][j] for j in range(4)]

    g0_m = np.zeros((128, 5 * npair), f)
    g0_mb2 = np.zeros((128, 5 * npair), f)
    g0_bn = np.zeros((128, 5 * npair), f)
    for b in range(5):
        bv = np.asarray(b2_g0[b], f)
        for j in range(npair):
            col = b * npair + j
            for half, s in ((0, 2 * j), (64, 2 * j + 1)):
                m = pol[s, b]
                g0_m[half:half + 64, col] = m
                g0_mb2[half:half + 64, col] = m * bv
                g0_bn[half:half + 64, col] = 1 - m
    d["g0_m"], d["g0_mb2"], d["g0_bn"] = g0_m, g0_mb2, g0_bn

    g1_m = np.zeros((128, 5 * n), f)
    g1_mb2 = np.zeros((128, 5 * n), f)
    g1_bn = np.zeros((128, 5 * n), f)
    for b in range(5):
        bv = np.asarray(b2_g1[b], f)
        for s in range(n):
            m = pol[s, 5 + b]
            j = b * n + s
            g1_m[:, j], g1_mb2[:, j], g1_bn[:, j] = m, m * bv, 1 - m
    d["g1_m"], d["g1_mb2"], d["g1_bn"] = g1_m, g1_mb2, g1_bn

    g2_m = np.zeros((128, 5 * n), f)
    g2_mb2 = np.zeros((128, 2 * 5 * n), f)
    g2_bn = np.zeros((128, 5 * n), f)
    for b in range(5):
        bv = np.asarray(b2_g2[b], f)
        for s in range(n):
            m = pol[s, 10 + b]
            j = b * n + s
            g2_m[:, j], g2_bn[:, j] = m, 1 - m
            for mo in range(2):
                g2_mb2[:, 2 * j + mo] = m * bv[mo * 128:(mo + 1) * 128]
    d["g2_m"], d["g2_mb2"], d["g2_bn"] = g2_m, g2_mb2, g2_bn

    d["fcb"] = np.tile(inputs["fc_b"].astype(f).reshape(1, 1000), (n, 1))
    return d


_program_cache = {}
TRACE = False   # set by test harness to capture an NTFF profile
LAST = None     # BassKernelResults of the last run


def kernel(**inputs):
    global LAST
    B = inputs["inputs"].shape[0]
    assert B % NCORES == 0
    n = B // NCORES
    npair = n // 2
    policy = (np.asarray(inputs["probs"]) >= 0.5)
    A = optimize_assignment(policy, NCORES, n)

    polA = policy[A]  # [cores, n, 15]
    ku0 = tuple(tuple(bool(polA[:, 2 * j:2 * j + 2, b].any())
                      for b in range(5)) for j in range(npair))
    ka0 = tuple(tuple(bool(polA[:, 2 * j:2 * j + 2, b].all())
                      for b in range(5)) for j in range(npair))
    ku1 = tuple(tuple(bool(polA[:, s, 5 + b].any()) for b in range(5))
                for s in range(n))
    ka1 = tuple(tuple(bool(polA[:, s, 5 + b].all()) for b in range(5))
                for s in range(n))
    ku2 = tuple(tuple(bool(polA[:, s, 10 + b].any()) for b in range(5))
                for s in range(n))
    ka2 = tuple(tuple(bool(polA[:, s, 10 + b].all()) for b in range(5))
                for s in range(n))

    key = (n, ku0, ka0, ku1, ka1, ku2, ka2)
    if key not in _program_cache:
        _program_cache.clear()
        _program_cache[key] = build_program(n, ku0, ka0, ku1, ka1, ku2, ka2)
    nc = _program_cache[key]
    shared = _prep_shared(inputs)
    polf = policy.astype(np.float32)
    in_maps = [_prep_core(inputs, shared, polf, A, c, n) for c in range(NCORES)]
    res = run_bass_kernel_spmd(nc, in_maps, core_ids=list(range(NCORES)),
                               trace=TRACE)
    LAST = res
    globals()["LAST_NC"] = nc
    globals()["LAST_IN_MAPS"] = in_maps
    out = np.zeros((B, 1000), np.float32)
    for c in range(NCORES):
        out[A[c]] = res.results[c]["y"]
    return out
